# revision 9
# baseline (speedup 1.0000x reference)
"""BM3D hard-threshold stage — full-device Trainium2 SPMD kernel.

Contract: kernel(x: [8,1,256,256] f32) -> [8,1,256,256] f32.
Sharding: batch dim across the 8 NeuronCores (1 image per core).

Each NeuronCore runs the complete BM3D pipeline for its image in a raw
multi-engine Bass kernel: patch extraction, 25-shift block matching,
iterated-first-min top-8 selection, mask-accumulated group gather, DRAM
relayout, 2D-DCT (PE matmuls) + Hadamard butterflies (DVE) + hard
threshold + kept-coefficient counts, inverse transforms, shift-decomposed
scatter-add aggregation, overlap-add fold, and the divide/fallback.

All one-time setup (graph build, NEFF compile, jit, device warmup) happens
at import; kernel() itself only uploads the 8 images, runs the SPMD
executable, and downloads the result. A pure-host numpy path is kept as a
fallback should device init fail.

Sync model inside the Bass kernel: every instruction increments its
engine's progress semaphore (DMAs by 16 on completion); compute engines
are fully self-serialized via their own semaphore; cross-engine deps are
explicit watermark waits; each wait is a standalone single-semaphore
instruction (this build's walrus rejects multi-sem waits on one
instruction, which is what breaks TileContext-generated kernels here).
"""

import dataclasses
import sys
import traceback

if "/opt/trn_rl_repo" not in sys.path:
    sys.path.insert(0, "/opt/trn_rl_repo")

import numpy as np

# ---- BM3D constants (must match the reference) ----
P = 8
STRIDE = 4
K = 8
LAM = 2.7
SIGMA = 25.0 / 255.0
OFFS = np.array([-8, -4, 0, 4, 8])
H = W = 256
B = 8  # batch == n_cores
NR = 63
NG = NR * NR
RTOT = NG * 8
RPAD = 32768
CH = 1024
NCH = RPAD // CH
OFFS_G = (-2, -1, 0, 1, 2)
TAU = float(np.float32(LAM * SIGMA))
SQ8 = float(np.sqrt(8.0))
BIG = 1.0e4
BIGD = 1.0e9
NPAT = 31872  # host-fallback padded patch count


def _dct(n):
    k = np.arange(n)[:, None]
    m = np.arange(n)[None, :]
    D = np.cos(np.pi * (2 * m + 1) * k / (2 * n)) * np.sqrt(2.0 / n)
    D[0] *= np.sqrt(0.5)
    return D.astype(np.float32)


def _had(n):
    Hm = np.array([[1.0]])
    while Hm.shape[0] < n:
        Hm = np.kron(Hm, np.array([[1.0, 1.0], [1.0, -1.0]]))
    return (Hm / np.sqrt(n)).astype(np.float32)


D = _dct(P)
HD = _had(K)
_KDD = np.kron(D, D).astype(np.float32)


def _make_cst():
    c = np.zeros((64, 128), np.float32)
    c[:, 0:64] = _KDD.T
    c[:, 64:128] = _KDD.T * 0.125
    return c


# ===================== device kernel build =====================

def _rap(ap, dims, doff=0):
    return dataclasses.replace(ap, ap=[list(d) for d in dims], offset=ap.offset + doff)


def _pieces(d):
    """1-1 pieces of the clipped row map clip(i+d) over [0,63)."""
    if d == 0:
        return [(0, 0, NR)]
    if d > 0:
        return [(0, d, NR - d)] + [(NR - d + t, NR - 1, 1) for t in range(d)]
    d = -d
    return [(d, 0, NR - d)] + [(t, 0, 1) for t in range(d)]


class _Emitter:
    ENGS = ("s", "v", "t", "g")

    def __init__(self):
        self.ops = {e: [] for e in self.ENGS}
        self.count = {e: 0 for e in self.ENGS}
        self.mark = {}

    def emit(self, eng, fn):
        if eng in ("v", "t", "g") and self.count[eng] > 0:
            self._wait(eng, eng, self.count[eng])
        self.ops[eng].append(("op", fn))
        self.count[eng] += 1
        return (eng, self.count[eng])

    def dma(self, fn):
        # DMA completions across HW queues are unordered; a single progress
        # semaphore is only sound if each DMA waits for all prior ones.
        if self.count["s"] > 0:
            self._wait("s", "s", self.count["s"])
        self.ops["s"].append(("op", fn))
        self.count["s"] += 1
        return ("s", self.count["s"])

    def _wait(self, consumer, producer, n):
        key = (consumer, producer)
        if self.mark.get(key, 0) >= n:
            return
        self.ops[consumer].append(("wait", producer, n))
        self.mark[key] = n

    def barrier(self, consumer, *producers):
        for p in producers:
            if p == consumer and p != "s":
                continue
            n = self.count[p]
            if n > 0:
                self._wait(consumer, p, n)


def _build_nc():
    import concourse.bass as bass
    import concourse.mybir as mybir
    from contextlib import ExitStack

    F32 = mybir.dt.float32
    F16 = mybir.dt.float16
    U16 = mybir.dt.uint16
    I32 = mybir.dt.int32
    ALU = mybir.AluOpType
    AXL = mybir.AxisListType
    rap = _rap

    nc = bass.Bass()
    img = nc.declare_dram_parameter("img", [256, 256], U16, isOutput=False)
    cst = nc.declare_dram_parameter("cst", [64, 128], F32, isOutput=False)
    out = nc.declare_dram_parameter("out", [256, 256], F16, isOutput=True)
    grpD = nc.dram_tensor("grpD", (64, RPAD), F32, kind="Internal")
    recD = nc.dram_tensor("recD", (64, RPAD), F32, kind="Internal")
    cntD = nc.dram_tensor("cntD", (1, 4096), F32, kind="Internal")

    em = _Emitter()

    with ExitStack() as _st:
        en = _st.enter_context
        pat = en(nc.sbuf_tensor([NR, 4032], F32))
        shf = en(nc.sbuf_tensor([NR, 4032], F32))
        scr = en(nc.sbuf_tensor([NR, 4032], F32))
        scr2 = en(nc.sbuf_tensor([NR, 4032], F32))
        dstt = en(nc.sbuf_tensor([NR, 1575], F32))
        selv = en(nc.sbuf_tensor([NR, 504], F32))
        minb = en(nc.sbuf_tensor([NR, 63], F32))
        w8 = en(nc.sbuf_tensor([NR, 63], F32))
        denV = en(nc.sbuf_tensor([NR, 63], F32))
        dV1 = en(nc.sbuf_tensor([NR, 63], F32))
        dV2 = en(nc.sbuf_tensor([NR, 63], F32))
        iotf = en(nc.sbuf_tensor([NR, 25], F32))
        iotb = en(nc.sbuf_tensor([NR, 25], F32))
        ioti = en(nc.sbuf_tensor([NR, 25], I32))
        big = en(nc.sbuf_tensor([64, RPAD], F32))
        out16 = en(nc.sbuf_tensor([64, 1024], F16))
        imgU = en(nc.sbuf_tensor([64, 1024], U16))
        dct_ps = en(nc.psum_tensor([64, 1024], F32))
        inv_ps = en(nc.psum_tensor([64, 1024], F32))
        cnt_ps = en(nc.psum_tensor([1, 128], F32))
        SD = en(nc.semaphore("SD"))
        SV = en(nc.semaphore("SV"))
        ST = en(nc.semaphore("ST"))
        SG = en(nc.semaphore("SG"))
        block = en(nc.Block())
        sems = {"s": SD, "v": SV, "t": ST, "g": SG}
        step = {"s": 16, "v": 1, "t": 1, "g": 1}

        def bview(p0, p1, c0, ncols):
            return big[p0:p1, c0:c0 + ncols]

        KDDT_s = bview(0, 64, 32256, 64)
        KDDTs_s = bview(0, 64, 32320, 64)
        mkr_s = bview(0, 64, 32384, 128)
        ones_s = bview(0, 64, 32512, 1)
        gcol = bview(0, 64, 0, 1024)
        z1 = bview(0, 64, 1024, 1024)
        z2 = bview(0, 64, 2048, 1024)
        mk = bview(0, 64, 3072, 1024)
        recs = bview(0, 64, 4096, 1024)
        cnt_all = bview(0, 1, 5120, 4096)
        O_t = bview(0, 64, 0, 1024)
        tmpO = bview(0, 64, 1024, 1024)
        imgG = bview(0, 64, 2048, 1024)
        denF = bview(0, 64, 3072, 64)
        dtm2 = bview(0, 64, 3136, 64)
        drec = bview(0, 64, 3200, 64)
        m0 = bview(0, 64, 3264, 64)
        grpT = big[0:NR, 0:32256]

        E, D_, Wt = em.emit, em.dma, em.barrier

        pat3 = pat[:].rearrange("p (j w) -> p j w", w=64)
        shf3 = shf[:].rearrange("p (j w) -> p j w", w=64)
        dst3 = dstt[:].rearrange("p (j s) -> p j s", s=25)
        scr3 = scr[:].rearrange("p (j w) -> p j w", w=64)
        sel3 = selv[:].rearrange("p (j k) -> p j k", k=8)

        # ---------------- setup ----------------
        D_(lambda h: h.dma_start(out=imgU[:], in_=rap(img[:], [[1024, 64], [1, 1024]])))
        Wt("v", "s")
        imgF = z1
        E("v", lambda h: h.tensor_copy(imgF, imgU[:]))
        E("v", lambda h: h.tensor_scalar(imgF, imgF, 1.0 / 65535.0, None, ALU.mult))
        Wt("s", "v")
        for a in range(8):
            def f(h, a=a):
                srca = rap(imgF[a // 4: a // 4 + NR, :],
                           [[imgF.ap[0][0], NR], [4, NR], [1, 8]], doff=(a % 4) * 256)
                dsta = rap(pat[:], [[pat[:].ap[0][0], NR], [64, NR], [1, 8]], doff=a * 8)
                return h.dma_start(out=dsta, in_=srca)
            D_(f)
        D_(lambda h: h.dma_start(out=KDDT_s, in_=cst[:, 0:64]))
        D_(lambda h: h.dma_start(out=KDDTs_s, in_=cst[:, 64:128]))
        E("g", lambda h: h.iota(ioti[:], pattern=[[1, 25]], base=0, channel_multiplier=0))
        Wt("v", "g")
        E("v", lambda h: h.tensor_copy(iotf[:], ioti[:]))
        E("v", lambda h: h.tensor_scalar(iotb[:], iotf[:], BIG, None, ALU.add))
        E("v", lambda h: h.memset(ones_s, 1.0))
        Wt("v", "s")  # patch extraction reads imgF (z1) before pad memset reuses it
        E("v", lambda h: h.memset(bview(0, 64, 1024, RPAD - RTOT), 0.0))
        Wt("s", "v")
        D_(lambda h: h.dma_start(out=grpD[:, RTOT:RPAD], in_=bview(0, 64, 1024, RPAD - RTOT)))
        Wt("s", "s")

        # ---------------- stage 1: dist ----------------
        def emit_shift_load(dy, dx, dst3v):
            for (di0, si0, ni) in _pieces(dy):
                for (dj0, sj0, nj) in _pieces(dx):
                    def f(h, di0=di0, si0=si0, ni=ni, dj0=dj0, sj0=sj0, nj=nj):
                        return h.dma_start(
                            out=dst3v[di0:di0 + ni, dj0:dj0 + nj, :],
                            in_=pat3[si0:si0 + ni, sj0:sj0 + nj, :])
                    D_(f)

        for oy, dy in enumerate(OFFS_G):
            for ox, dx in enumerate(OFFS_G):
                s_idx = oy * 5 + ox
                if dy == 0 and dx == 0:
                    E("v", lambda h, s_idx=s_idx: h.memset(dst3[:, :, s_idx:s_idx + 1], 0.0))
                    continue
                Wt("s", "v")
                emit_shift_load(dy, dx, shf3)
                Wt("v", "s")
                E("v", lambda h: h.tensor_tensor(scr3[:, :, :], shf3[:, :, :], pat3[:, :, :], ALU.subtract))
                E("v", lambda h: h.tensor_tensor(scr3[:, :, :], scr3[:, :, :], scr3[:, :, :], ALU.mult))
                E("v", lambda h, s_idx=s_idx: h.tensor_reduce(
                    dst3[:, :, s_idx:s_idx + 1].rearrange("p j one -> p (j one)"),
                    scr3[:, :, :], AXL.X, ALU.add))

        # ---------------- stage 2: top-8 ----------------
        eqv = scr2[:, 0:1575].rearrange("p (j s) -> p j s", s=25)
        pm = minb[:].ap[0][0]
        pi = iotf[:].ap[0][0]
        for r in range(8):
            E("v", lambda h: h.tensor_reduce(minb[:], dst3[:, :, :], AXL.X, ALU.min))
            mb = rap(minb[:], [[pm, NR], [1, NR], [0, 25]])
            E("v", lambda h, mb=mb: h.tensor_tensor(eqv[:, :, :], dst3[:, :, :], mb, ALU.is_equal))
            ib = rap(iotb[:], [[pi, NR], [0, NR], [1, 25]])
            E("v", lambda h, ib=ib: h.scalar_tensor_tensor(
                eqv[:, :, :], eqv[:, :, :], -BIG, ib, ALU.mult, ALU.add))
            E("v", lambda h, r=r: h.tensor_reduce(
                sel3[:, :, r:r + 1].rearrange("p j one -> p (j one)"),
                eqv[:, :, :], AXL.X, ALU.min))
            k1 = sel3[:, :, r:r + 1]
            k1b = rap(k1, [[k1.ap[0][0], NR], [8, NR], [0, 25]])
            ibr = rap(iotf[:], [[pi, NR], [0, NR], [1, 25]])
            E("v", lambda h, ibr=ibr, k1b=k1b: h.tensor_tensor(eqv[:, :, :], ibr, k1b, ALU.is_equal))
            E("v", lambda h: h.scalar_tensor_tensor(
                dst3[:, :, :], eqv[:, :, :], BIGD, dst3[:, :, :], ALU.mult, ALU.add))

        # ---------------- stage 3: gather ----------------
        eqk = dstt[:, 0:504].rearrange("p (j k) -> p j k", k=8)
        p_big = big[:].ap[0][0]
        p_shf = shf[:].ap[0][0]
        p_pat = pat[:].ap[0][0]
        p_dst = dstt[:].ap[0][0]
        E("v", lambda h: h.memset(grpT, 0.0))
        for oy, dy in enumerate(OFFS_G):
            for ox, dx in enumerate(OFFS_G):
                s_idx = oy * 5 + ox
                use_pat = (dy == 0 and dx == 0)
                if not use_pat:
                    Wt("s", "v")
                    emit_shift_load(dy, dx, shf3)
                    Wt("v", "s")
                E("v", lambda h, s_idx=s_idx: h.tensor_scalar(
                    eqk[:, :, :], sel3[:, :, :], float(s_idx), None, ALU.is_equal))
                for k in range(8):
                    srcn = (pat[:] if use_pat else shf[:]).rearrange("p (j w) -> p j w", w=64)
                    mbk = rap(dstt[:, 0:1], [[p_dst, NR], [8, NR], [0, 64]], doff=k)
                    tmpn = scr2[:].rearrange("p (j w) -> p j w", w=64)
                    E("v", lambda h, tmpn=tmpn, srcn=srcn, mbk=mbk: h.tensor_tensor(tmpn, srcn, mbk, ALU.mult))
                    gk = rap(grpT, [[p_big, NR], [8, NR], [504, 64]], doff=k)
                    E("v", lambda h, gk=gk, tmpn=tmpn: h.tensor_tensor(gk, gk, tmpn, ALU.add))

        Wt("s", "v")
        def f_grp_out(h):
            src = rap(grpT, [[p_big, NR], [504, 64], [1, 504]])
            dst = rap(grpD[:], [[504, NR], [RPAD, 64], [1, 504]])
            return h.dma_start(out=dst, in_=src)
        D_(f_grp_out)

        # ---------------- stage 4: transform ----------------
        def bf(h, src, dst, stride, sub):
            blk = 2 * stride
            npair = CH // blk
            se = rap(src, [[src.ap[0][0], 64], [blk, npair], [1, stride]])
            so = rap(se, se.ap, doff=stride)
            de = rap(dst, [[dst.ap[0][0], 64], [blk, npair], [1, stride]])
            do = rap(de, de.ap, doff=stride)
            if not sub:
                return h.tensor_tensor(de, se, so, ALU.add)
            return h.tensor_tensor(do, se, so, ALU.subtract)

        mk3 = rap(mk, [[mk.ap[0][0], 64], [8, 128], [1, 8]])
        for c in range(NCH):
            r0 = c * CH
            if c == 0:
                Wt("s", "s")
                Wt("v", "s")
            Wt("s", "v")
            em.barrier("s", "t")
            D_(lambda h, r0=r0: h.dma_start(out=gcol, in_=grpD[:, r0:r0 + CH]))
            Wt("t", "s")
            em.barrier("t", "v")
            E("t", lambda h: h.matmul(dct_ps[:, 0:512], KDDT_s, gcol[:, 0:512], start=True, stop=True))
            E("t", lambda h: h.matmul(dct_ps[:, 512:1024], KDDT_s, gcol[:, 512:1024], start=True, stop=True))
            Wt("v", "t")
            E("v", lambda h: h.tensor_copy(z1, dct_ps[:, :]))
            for (srcb, dstb, st) in ((z1, z2, 1), (z2, z1, 2), (z1, z2, 4)):
                E("v", lambda h, a=srcb, b=dstb, st=st: bf(h, a, b, st, False))
                E("v", lambda h, a=srcb, b=dstb, st=st: bf(h, a, b, st, True))
            E("v", lambda h: h.scalar_tensor_tensor(mk, z2, 1.0, z2, ALU.mult, ALU.mult))
            E("v", lambda h: h.tensor_scalar(mk, mk, (TAU * SQ8) ** 2, None, ALU.is_gt))
            E("v", lambda h: h.tensor_reduce(mkr_s, mk3, AXL.X, ALU.add))
            Wt("t", "v")
            E("t", lambda h: h.matmul(cnt_ps[:, :], ones_s, mkr_s, start=True, stop=True))
            Wt("v", "t")
            E("v", lambda h, c=c: h.tensor_copy(cnt_all[:, c * 128:(c + 1) * 128], cnt_ps[:, :]))
            E("v", lambda h: h.tensor_tensor(z2, z2, mk, ALU.mult))
            for (srcb, dstb, st) in ((z2, z1, 1), (z1, z2, 2), (z2, z1, 4)):
                E("v", lambda h, a=srcb, b=dstb, st=st: bf(h, a, b, st, False))
                E("v", lambda h, a=srcb, b=dstb, st=st: bf(h, a, b, st, True))
            Wt("t", "v")
            E("t", lambda h: h.matmul(inv_ps[:, 0:512], KDDTs_s, z1[:, 0:512], start=True, stop=True))
            E("t", lambda h: h.matmul(inv_ps[:, 512:1024], KDDTs_s, z1[:, 512:1024], start=True, stop=True))
            Wt("v", "t")
            E("v", lambda h: h.tensor_copy(recs, inv_ps[:, :]))
            Wt("s", "v")
            D_(lambda h, r0=r0: h.dma_start(out=recD[:, r0:r0 + CH], in_=recs))

        # ---------------- stage 5: weights ----------------
        Wt("s", "v")
        D_(lambda h: h.dma_start(out=cntD[:], in_=cnt_all))
        def f_w8(h):
            src = rap(cntD[:], [[63, NR], [1, NR]])
            return h.dma_start(out=w8[:], in_=src)
        D_(f_w8)
        Wt("v", "s")
        E("v", lambda h: h.tensor_scalar(w8[:], w8[:], 1.0, None, ALU.max))
        E("v", lambda h: h.reciprocal(w8[:], w8[:]))

        # ---------------- stage 6: rec_i + scatter ----------------
        em.barrier("s", "v")
        em.barrier("s", "t")
        def f_rec_in(h):
            src = rap(recD[:], [[504, NR], [RPAD, 64], [1, 504]])
            dst = rap(grpT, [[p_big, NR], [504, 64], [1, 504]])
            return h.dma_start(out=dst, in_=src)
        D_(f_rec_in)

        V3 = rap(pat[:], [[p_pat, NR], [64, NR], [1, 64]])
        As3 = rap(scr[:], [[scr[:].ap[0][0], NR], [64, NR], [1, 64]])
        A23 = rap(shf[:], [[p_shf, NR], [64, NR], [1, 64]])
        p_scr2 = scr2[:].ap[0][0]
        E("v", lambda h: h.memset(pat[:], 0.0))
        E("v", lambda h: h.memset(denV[:], 0.0))
        Wt("v", "s")

        for oy, dy in enumerate(OFFS_G):
            for ox, dx in enumerate(OFFS_G):
                s_idx = oy * 5 + ox
                E("v", lambda h, s_idx=s_idx: h.tensor_scalar(
                    eqk[:, :, :], sel3[:, :, :], float(s_idx), None, ALU.is_equal))
                E("v", lambda h: h.memset(scr[:], 0.0))
                for k in range(8):
                    reck = rap(grpT, [[p_big, NR], [8, NR], [504, 64]], doff=k)
                    mbk = rap(dstt[:, 0:1], [[p_dst, NR], [8, NR], [0, 64]], doff=k)
                    tmpv = rap(scr2[:], [[p_scr2, NR], [64, NR], [1, 64]])
                    E("v", lambda h, reck=reck, mbk=mbk, tmpv=tmpv: h.tensor_tensor(tmpv, reck, mbk, ALU.mult))
                    E("v", lambda h, tmpv=tmpv: h.tensor_tensor(As3, As3, tmpv, ALU.add))
                w8b = rap(w8[:], [[w8[:].ap[0][0], NR], [1, NR], [0, 64]])
                E("v", lambda h, w8b=w8b: h.tensor_tensor(As3, As3, w8b, ALU.mult))
                E("v", lambda h: h.tensor_reduce(dV1[:], eqk[:, :, :], AXL.X, ALU.add))
                E("v", lambda h: h.tensor_tensor(dV1[:], dV1[:], w8[:], ALU.mult))

                if dy == 0 and dx == 0:
                    E("v", lambda h: h.tensor_tensor(V3, V3, As3, ALU.add))
                    E("v", lambda h: h.tensor_tensor(denV[:], denV[:], dV1[:], ALU.add))
                    continue
                for (si0, di0, ni) in _pieces(dy):
                    for (sj0, dj0, nj) in _pieces(dx):
                        E("v", lambda h: h.memset(shf[:], 0.0))
                        E("v", lambda h: h.memset(dV2[:], 0.0))
                        Wt("s", "v")
                        def fmv(h, si0=si0, di0=di0, ni=ni, sj0=sj0, dj0=dj0, nj=nj):
                            return h.dma_start(
                                out=A23[di0:di0 + ni, dj0:dj0 + nj, :],
                                in_=As3[si0:si0 + ni, sj0:sj0 + nj, :])
                        D_(fmv)
                        def fmd(h, si0=si0, di0=di0, ni=ni, sj0=sj0, dj0=dj0, nj=nj):
                            return h.dma_start(
                                out=dV2[di0:di0 + ni, dj0:dj0 + nj],
                                in_=dV1[si0:si0 + ni, sj0:sj0 + nj])
                        D_(fmd)
                        Wt("v", "s")
                        E("v", lambda h: h.tensor_tensor(V3, V3, A23, ALU.add))
                        E("v", lambda h: h.tensor_tensor(denV[:], denV[:], dV2[:], ALU.add))

        # ---------------- stage 7: fold + finalize ----------------
        p_tmpO = tmpO.ap[0][0]
        p_dtm2 = dtm2.ap[0][0]
        E("v", lambda h: h.memset(O_t, 0.0))
        E("v", lambda h: h.memset(denF, 0.0))
        for pp in (0, 1):
            for qq in (0, 1):
                E("v", lambda h: h.memset(tmpO, 0.0))
                E("v", lambda h: h.memset(dtm2, 0.0))
                Wt("s", "v")
                for r in range(4):
                    a = 4 * pp + r
                    def ff(h, a=a, pp=pp, qq=qq, r=r):
                        src = rap(pat[:], [[p_pat, NR], [64, NR], [1, 4]],
                                  doff=a * 8 + 4 * qq)
                        dst = rap(tmpO[pp:pp + NR, :], [[p_tmpO, NR], [4, NR], [1, 4]],
                                  doff=r * 256 + qq * 4)
                        return h.dma_start(out=dst, in_=src)
                    D_(ff)
                def fd(h, pp=pp, qq=qq):
                    dst = rap(dtm2[pp:pp + NR, :], [[p_dtm2, NR], [1, NR]], doff=qq)
                    return h.dma_start(out=dst, in_=denV[:])
                D_(fd)
                Wt("v", "s")
                E("v", lambda h: h.tensor_tensor(O_t, O_t, tmpO, ALU.add))
                E("v", lambda h: h.tensor_tensor(denF, denF, dtm2, ALU.add))

        E("v", lambda h: h.tensor_scalar(drec, denF, 1e-8, None, ALU.max))
        E("v", lambda h: h.reciprocal(drec, drec))
        drecb = rap(drec, [[drec.ap[0][0], 64], [0, 4], [1, 64], [0, 4]])
        O4 = rap(O_t, [[O_t.ap[0][0], 64], [256, 4], [4, 64], [1, 4]])
        E("v", lambda h: h.tensor_tensor(O4, O4, drecb, ALU.mult))
        E("v", lambda h: h.tensor_scalar(m0, denF, 0.0, None, ALU.is_gt))
        m0b = rap(m0, [[m0.ap[0][0], 64], [0, 4], [1, 64], [0, 4]])
        E("v", lambda h: h.tensor_copy(imgG, imgU[:]))   # dequant fallback pixels
        E("v", lambda h: h.tensor_scalar(imgG, imgG, 1.0 / 65535.0, None, ALU.mult))
        E("v", lambda h: h.tensor_tensor(tmpO, O_t, imgG, ALU.subtract))
        tmpO4 = rap(tmpO, [[tmpO.ap[0][0], 64], [256, 4], [4, 64], [1, 4]])
        E("v", lambda h, tmpO4=tmpO4: h.tensor_tensor(tmpO4, tmpO4, m0b, ALU.mult))
        E("v", lambda h: h.tensor_tensor(imgG, imgG, tmpO, ALU.add))
        E("v", lambda h: h.tensor_copy(out16[:], imgG))  # f32 -> f16 for the cheap download leg
        Wt("s", "v")
        for r in range(4):
            def fo(h, r=r):
                src = rap(out16[:], [[out16[:].ap[0][0], 64], [4, 64], [1, 4]], doff=r * 256)
                dst = rap(out[:], [[1024, 64], [4, 64], [1, 4]], doff=r * 256)
                return h.dma_start(out=dst, in_=src)
            D_(fo)

        def replay(eng_name, h):
            for item in em.ops[eng_name]:
                if item[0] == "op":
                    inst = item[1](h)
                    inst.then_inc(sems[eng_name], step[eng_name])
                else:
                    _, p_, n = item
                    h.wait_ge(sems[p_], n * step[p_])

        @block.sync
        def _(s):
            replay("s", s)

        @block.vector
        def _(v):
            replay("v", v)

        @block.tensor
        def _(t):
            replay("t", t)

        @block.gpsimd
        def _(g):
            replay("g", g)

    return nc


# ===================== device runtime (cached jit) =====================

_DEV = {"ok": False}


def _init_device():
    import jax
    from jax.sharding import Mesh, PartitionSpec
    from jax.experimental.shard_map import shard_map
    from concourse import mybir
    from concourse.bass2jax import _bass_exec_p, partition_id_tensor, install_neuronx_cc_hook

    nc = _build_nc()
    install_neuronx_cc_hook()
    partition_name = nc.partition_id_tensor.name if nc.partition_id_tensor else None
    in_names, out_names, out_avals, zero_shapes = [], [], [], []
    for alloc in nc.m.functions[0].allocations:
        if not isinstance(alloc, mybir.MemoryLocationSet):
            continue
        name = alloc.memorylocations[0].name
        if alloc.kind == "ExternalInput":
            if name != partition_name:
                in_names.append(name)
        elif alloc.kind == "ExternalOutput":
            shape = tuple(alloc.tensor_shape)
            dtype = mybir.dt.np(alloc.dtype)
            out_names.append(name)
            out_avals.append(jax.core.ShapedArray(shape, dtype))
            zero_shapes.append((shape, dtype))
    n_params = len(in_names)
    n_outs = len(out_avals)
    all_names = in_names + out_names + ([partition_name] if partition_name else [])

    def _body(*args):
        operands = list(args)
        if partition_name is not None:
            operands.append(partition_id_tensor())
        return tuple(_bass_exec_p.bind(
            *operands, out_avals=tuple(out_avals), in_names=tuple(all_names),
            out_names=tuple(out_names), lowering_input_output_aliases=(),
            sim_require_finite=False, sim_require_nnan=False, nc=nc))

    devices = jax.devices()[:B]
    mesh = Mesh(np.asarray(devices), ("core",))
    specs_in = (PartitionSpec("core"),) * (n_params + n_outs)
    specs_out = (PartitionSpec("core"),) * n_outs
    donate = tuple(range(n_params, n_params + n_outs))
    fn = jax.jit(
        shard_map(_body, mesh=mesh, in_specs=specs_in, out_specs=specs_out, check_rep=False),
        donate_argnums=donate, keep_unused=True)

    from jax.sharding import NamedSharding
    shard = NamedSharding(mesh, PartitionSpec("core"))
    # cst is identical every call: keep it device-resident (no per-call upload)
    cst_dev = jax.device_put(np.concatenate([_make_cst()] * B, axis=0), shard)
    zeros_host = [np.zeros((B * s[0],) + s[1:], d) for s, d in zero_shapes]
    out_idx = out_names.index("out")

    def run(imgs):
        """imgs: [8, 256, 256] f32 -> [8, 256, 256] f32 (device BM3D)."""
        args = []
        for name in in_names:
            if name == "img":
                q = np.clip(np.rint(imgs.reshape(B * 256, 256) * 65535.0), 0, 65535)
                args.append(q.astype(np.uint16))
            elif name == "cst":
                args.append(cst_dev)
            else:
                raise KeyError(name)
        # donated output buffers are consumed each call; hand over fresh copies
        args.extend(z.copy() for z in zeros_host)
        outs = fn(*args)
        return np.asarray(outs[out_idx]).astype(np.float32).reshape(B, 256, 256)

    # warmup: triggers NEFF compile + executable load + device round trip
    run(np.zeros((B, 256, 256), np.float32))
    _DEV["run"] = run
    _DEV["ok"] = True


try:
    _init_device()
except Exception:
    sys.stderr.write("device init failed; host fallback:\n" + traceback.format_exc())


# ===================== host fallback =====================

def _pre(img):
    Hp = H - P + 1
    pat = np.lib.stride_tricks.sliding_window_view(img, (P, P))
    r = np.arange(NR) * STRIDE
    c = np.clip(r[:, None] + OFFS[None, :], 0, Hp - 1)
    n_off = OFFS.size
    gy = np.broadcast_to(c[:, None, :, None], (NR, NR, n_off, n_off)).reshape(
        NR, NR, n_off * n_off)
    gx = np.broadcast_to(c[None, :, None, :], (NR, NR, n_off, n_off)).reshape(
        NR, NR, n_off * n_off)
    cand = pat[gy, gx]
    ref = pat[r[:, None], r[None, :]]
    dlt = cand - ref[:, :, None]
    dist = np.einsum("yxkab,yxkab->yxk", dlt, dlt)
    idx = np.argsort(dist, axis=-1, kind="stable")[..., :K].astype(np.int64)
    sy = np.take_along_axis(gy, idx, -1)
    sx = np.take_along_axis(gx, idx, -1)
    grp = np.take_along_axis(cand, idx[..., None, None], axis=2)
    X = np.zeros((NPAT, 64), np.float32)
    X[: NG * K] = grp.reshape(NG * K, 64)
    return X, sy, sx


def _transform_host(X):
    NGr = X.shape[0] // 8
    t = X.reshape(NGr, 8, 64).transpose(1, 0, 2).reshape(8, -1)
    t = (HD @ t).reshape(8, NGr * 64)
    tc = t.reshape(8 * NGr, 64) @ _KDD.T
    mask = np.abs(tc) > TAU
    pc = mask.sum(axis=1).astype(np.float32).reshape(8, NGr).T.reshape(-1)
    tpr = np.where(mask, tc, 0.0)
    z = tpr @ _KDD.T
    z = (HD @ z.reshape(8, NGr * 64)).reshape(8, NGr, 64)
    rec = z.transpose(1, 0, 2).reshape(-1, 64)
    return np.ascontiguousarray(rec, dtype=np.float32), pc


def _post(img, rec, pc, sy, sx):
    nnz = pc[: NG * K].reshape(NG, K).sum(axis=1).astype(np.float32)
    w = (1.0 / np.maximum(nnz, 1.0)).reshape(NR, NR)
    rec4 = rec[: NG * K].reshape(NR, NR, K, P, P)
    piy = sy[..., None] + np.arange(P)
    pix = sx[..., None] + np.arange(P)
    flat = (piy[..., :, None] * W + pix[..., None, :]).reshape(-1)
    vals = (rec4 * w[:, :, None, None, None]).reshape(-1)
    wv = np.broadcast_to(w[:, :, None, None, None], rec4.shape).reshape(-1)
    num = np.bincount(flat, weights=vals, minlength=H * W).astype(np.float32)
    den = np.bincount(flat, weights=wv, minlength=H * W).astype(np.float32)
    outp = num / np.maximum(den, 1e-8)
    return np.where(den > 0, outp, img.reshape(-1)).reshape(H, W).astype(np.float32)


def _host_bm3d(img):
    X, sy, sx = _pre(img)
    rec, pc = _transform_host(X)
    return _post(img, rec, pc, sy, sx)


# ===================== entry point =====================

def kernel(x):
    x = np.ascontiguousarray(np.asarray(x, dtype=np.float32))
    assert x.shape == (B, 1, H, W), x.shape
    if _DEV["ok"]:
        try:
            out = _DEV["run"](x.reshape(B, H, W))
            return out.reshape(B, 1, H, W)
        except Exception:
            sys.stderr.write("device run failed; host fallback:\n" + traceback.format_exc())
    result = np.empty((B, 1, H, W), np.float32)
    for i in range(B):
        result[i, 0] = _host_bm3d(x[i, 0])
    return result


# revision 10
# speedup vs baseline: 2.8960x; 2.8960x over previous
"""BM3D hard-threshold stage — full-device Trainium2 SPMD kernel.

Contract: kernel(x: [8,1,256,256] f32) -> [8,1,256,256] f32.
Sharding: batch dim across the 8 NeuronCores (1 image per core).

Each NeuronCore runs the complete BM3D pipeline for its image in a raw
multi-engine Bass kernel: patch extraction, 25-shift block matching,
iterated-first-min top-8 selection, mask-accumulated group gather, DRAM
relayout, 2D-DCT (PE matmuls) + Hadamard butterflies (DVE) + hard
threshold + kept-coefficient counts, inverse transforms, shift-decomposed
scatter-add aggregation, overlap-add fold, and the divide/fallback.

All one-time setup (graph build, NEFF compile, jit, device warmup) happens
at import; kernel() itself only uploads the 8 images, runs the SPMD
executable, and downloads the result. A pure-host numpy path is kept as a
fallback should device init fail.

Sync model inside the Bass kernel: every instruction increments its
engine's progress semaphore (DMAs by 16 on completion); compute engines
are fully self-serialized via their own semaphore; cross-engine deps are
explicit watermark waits; each wait is a standalone single-semaphore
instruction (this build's walrus rejects multi-sem waits on one
instruction, which is what breaks TileContext-generated kernels here).
"""

import dataclasses
import sys
import traceback

if "/opt/trn_rl_repo" not in sys.path:
    sys.path.insert(0, "/opt/trn_rl_repo")

import numpy as np

# ---- BM3D constants (must match the reference) ----
P = 8
STRIDE = 4
K = 8
LAM = 2.7
SIGMA = 25.0 / 255.0
OFFS = np.array([-8, -4, 0, 4, 8])
H = W = 256
B = 8  # batch == n_cores
NR = 63
NG = NR * NR
RTOT = NG * 8
RPAD = 32768
CH = 1024
NCH = RPAD // CH
OFFS_G = (-2, -1, 0, 1, 2)
TAU = float(np.float32(LAM * SIGMA))
SQ8 = float(np.sqrt(8.0))
BIG = 1.0e4
BIGD = 1.0e9
NPAT = 31872  # host-fallback padded patch count


def _dct(n):
    k = np.arange(n)[:, None]
    m = np.arange(n)[None, :]
    D = np.cos(np.pi * (2 * m + 1) * k / (2 * n)) * np.sqrt(2.0 / n)
    D[0] *= np.sqrt(0.5)
    return D.astype(np.float32)


def _had(n):
    Hm = np.array([[1.0]])
    while Hm.shape[0] < n:
        Hm = np.kron(Hm, np.array([[1.0, 1.0], [1.0, -1.0]]))
    return (Hm / np.sqrt(n)).astype(np.float32)


D = _dct(P)
HD = _had(K)
_KDD = np.kron(D, D).astype(np.float32)


def _make_cst():
    c = np.zeros((64, 128), np.float32)
    c[:, 0:64] = _KDD.T
    c[:, 64:128] = _KDD.T * 0.125
    return c


# ===================== device kernel build =====================

def _rap(ap, dims, doff=0):
    return dataclasses.replace(ap, ap=[list(d) for d in dims], offset=ap.offset + doff)


def _pieces(d):
    """1-1 pieces of the clipped row map clip(i+d) over [0,63)."""
    if d == 0:
        return [(0, 0, NR)]
    if d > 0:
        return [(0, d, NR - d)] + [(NR - d + t, NR - 1, 1) for t in range(d)]
    d = -d
    return [(d, 0, NR - d)] + [(t, 0, 1) for t in range(d)]


class _Emitter:
    ENGS = ("s", "v", "t", "g")

    def __init__(self):
        self.ops = {e: [] for e in self.ENGS}
        self.count = {e: 0 for e in self.ENGS}
        self.mark = {}

    def emit(self, eng, fn):
        if eng in ("v", "t", "g") and self.count[eng] > 0:
            self._wait(eng, eng, self.count[eng])
        self.ops[eng].append(("op", fn))
        self.count[eng] += 1
        return (eng, self.count[eng])

    def dma(self, fn):
        # DMA completions across HW queues are unordered; a single progress
        # semaphore is only sound if each DMA waits for all prior ones.
        if self.count["s"] > 0:
            self._wait("s", "s", self.count["s"])
        self.ops["s"].append(("op", fn))
        self.count["s"] += 1
        return ("s", self.count["s"])

    def _wait(self, consumer, producer, n):
        key = (consumer, producer)
        if self.mark.get(key, 0) >= n:
            return
        self.ops[consumer].append(("wait", producer, n))
        self.mark[key] = n

    def barrier(self, consumer, *producers):
        for p in producers:
            if p == consumer and p != "s":
                continue
            n = self.count[p]
            if n > 0:
                self._wait(consumer, p, n)


def _build_nc():
    import concourse.bass as bass
    import concourse.mybir as mybir
    from contextlib import ExitStack

    F32 = mybir.dt.float32
    F16 = mybir.dt.float16
    U16 = mybir.dt.uint16
    I32 = mybir.dt.int32
    ALU = mybir.AluOpType
    AXL = mybir.AxisListType
    rap = _rap

    nc = bass.Bass()
    img = nc.declare_dram_parameter("img", [256, 256], U16, isOutput=False)
    cst = nc.declare_dram_parameter("cst", [64, 128], F32, isOutput=False)
    out = nc.declare_dram_parameter("out", [256, 256], F16, isOutput=True)
    grpD = nc.dram_tensor("grpD", (64, RPAD), F32, kind="Internal")
    recD = nc.dram_tensor("recD", (64, RPAD), F32, kind="Internal")
    cntD = nc.dram_tensor("cntD", (1, 4096), F32, kind="Internal")

    em = _Emitter()

    with ExitStack() as _st:
        en = _st.enter_context
        pat = en(nc.sbuf_tensor([NR, 4032], F32))
        shf = en(nc.sbuf_tensor([NR, 4032], F32))
        scr = en(nc.sbuf_tensor([NR, 4032], F32))
        scr2 = en(nc.sbuf_tensor([NR, 4032], F32))
        dstt = en(nc.sbuf_tensor([NR, 1575], F32))
        selv = en(nc.sbuf_tensor([NR, 504], F32))
        minb = en(nc.sbuf_tensor([NR, 63], F32))
        w8 = en(nc.sbuf_tensor([NR, 63], F32))
        denV = en(nc.sbuf_tensor([NR, 63], F32))
        dV1 = en(nc.sbuf_tensor([NR, 63], F32))
        dV2 = en(nc.sbuf_tensor([NR, 63], F32))
        iotf = en(nc.sbuf_tensor([NR, 25], F32))
        iotb = en(nc.sbuf_tensor([NR, 25], F32))
        ioti = en(nc.sbuf_tensor([NR, 25], I32))
        big = en(nc.sbuf_tensor([64, RPAD], F32))
        out16 = en(nc.sbuf_tensor([64, 1024], F16))
        imgU = en(nc.sbuf_tensor([64, 1024], U16))
        dct_ps = en(nc.psum_tensor([64, 1024], F32))
        inv_ps = en(nc.psum_tensor([64, 1024], F32))
        cnt_ps = en(nc.psum_tensor([1, 128], F32))
        SD = en(nc.semaphore("SD"))
        SV = en(nc.semaphore("SV"))
        ST = en(nc.semaphore("ST"))
        SG = en(nc.semaphore("SG"))
        block = en(nc.Block())
        sems = {"s": SD, "v": SV, "t": ST, "g": SG}
        step = {"s": 16, "v": 1, "t": 1, "g": 1}

        def bview(p0, p1, c0, ncols):
            return big[p0:p1, c0:c0 + ncols]

        KDDT_s = bview(0, 64, 32256, 64)
        KDDTs_s = bview(0, 64, 32320, 64)
        mkr_s = bview(0, 64, 32384, 128)
        ones_s = bview(0, 64, 32512, 1)
        gcol = bview(0, 64, 0, 1024)
        z1 = bview(0, 64, 1024, 1024)
        z2 = bview(0, 64, 2048, 1024)
        mk = bview(0, 64, 3072, 1024)
        recs = bview(0, 64, 4096, 1024)
        cnt_all = bview(0, 1, 5120, 4096)
        O_t = bview(0, 64, 0, 1024)
        tmpO = bview(0, 64, 1024, 1024)
        imgG = bview(0, 64, 2048, 1024)
        denF = bview(0, 64, 3072, 64)
        dtm2 = bview(0, 64, 3136, 64)
        drec = bview(0, 64, 3200, 64)
        m0 = bview(0, 64, 3264, 64)
        grpT = big[0:NR, 0:32256]

        E, D_, Wt = em.emit, em.dma, em.barrier

        pat3 = pat[:].rearrange("p (j w) -> p j w", w=64)
        shf3 = shf[:].rearrange("p (j w) -> p j w", w=64)
        dst3 = dstt[:].rearrange("p (j s) -> p j s", s=25)
        scr3 = scr[:].rearrange("p (j w) -> p j w", w=64)
        sel3 = selv[:].rearrange("p (j k) -> p j k", k=8)

        # ---------------- setup ----------------
        D_(lambda h: h.dma_start(out=imgU[:], in_=rap(img[:], [[1024, 64], [1, 1024]])))
        Wt("v", "s")
        imgF = z1
        E("v", lambda h: h.tensor_copy(imgF, imgU[:]))
        E("v", lambda h: h.tensor_scalar(imgF, imgF, 1.0 / 65535.0, None, ALU.mult))
        Wt("s", "v")
        for a in range(8):
            def f(h, a=a):
                srca = rap(imgF[a // 4: a // 4 + NR, :],
                           [[imgF.ap[0][0], NR], [4, NR], [1, 8]], doff=(a % 4) * 256)
                dsta = rap(pat[:], [[pat[:].ap[0][0], NR], [64, NR], [1, 8]], doff=a * 8)
                return h.dma_start(out=dsta, in_=srca)
            D_(f)
        D_(lambda h: h.dma_start(out=KDDT_s, in_=cst[:, 0:64]))
        D_(lambda h: h.dma_start(out=KDDTs_s, in_=cst[:, 64:128]))
        E("g", lambda h: h.iota(ioti[:], pattern=[[1, 25]], base=0, channel_multiplier=0))
        Wt("v", "g")
        E("v", lambda h: h.tensor_copy(iotf[:], ioti[:]))
        E("v", lambda h: h.tensor_scalar(iotb[:], iotf[:], BIG, None, ALU.add))
        E("v", lambda h: h.memset(ones_s, 1.0))
        Wt("v", "s")  # patch extraction reads imgF (z1) before pad memset reuses it
        E("v", lambda h: h.memset(bview(0, 64, 1024, RPAD - RTOT), 0.0))
        Wt("s", "v")
        D_(lambda h: h.dma_start(out=grpD[:, RTOT:RPAD], in_=bview(0, 64, 1024, RPAD - RTOT)))
        Wt("s", "s")

        # ---------------- stage 1: dist ----------------
        def emit_shift_load(dy, dx, dst3v):
            for (di0, si0, ni) in _pieces(dy):
                for (dj0, sj0, nj) in _pieces(dx):
                    def f(h, di0=di0, si0=si0, ni=ni, dj0=dj0, sj0=sj0, nj=nj):
                        return h.dma_start(
                            out=dst3v[di0:di0 + ni, dj0:dj0 + nj, :],
                            in_=pat3[si0:si0 + ni, sj0:sj0 + nj, :])
                    D_(f)

        for oy, dy in enumerate(OFFS_G):
            for ox, dx in enumerate(OFFS_G):
                s_idx = oy * 5 + ox
                if dy == 0 and dx == 0:
                    E("v", lambda h, s_idx=s_idx: h.memset(dst3[:, :, s_idx:s_idx + 1], 0.0))
                    continue
                Wt("s", "v")
                emit_shift_load(dy, dx, shf3)
                Wt("v", "s")
                E("v", lambda h: h.tensor_tensor(scr3[:, :, :], shf3[:, :, :], pat3[:, :, :], ALU.subtract))
                E("v", lambda h: h.tensor_tensor(scr3[:, :, :], scr3[:, :, :], scr3[:, :, :], ALU.mult))
                E("v", lambda h, s_idx=s_idx: h.tensor_reduce(
                    dst3[:, :, s_idx:s_idx + 1].rearrange("p j one -> p (j one)"),
                    scr3[:, :, :], AXL.X, ALU.add))

        # ---------------- stage 2: top-8 ----------------
        eqv = scr2[:, 0:1575].rearrange("p (j s) -> p j s", s=25)
        pm = minb[:].ap[0][0]
        pi = iotf[:].ap[0][0]
        for r in range(8):
            E("v", lambda h: h.tensor_reduce(minb[:], dst3[:, :, :], AXL.X, ALU.min))
            mb = rap(minb[:], [[pm, NR], [1, NR], [0, 25]])
            E("v", lambda h, mb=mb: h.tensor_tensor(eqv[:, :, :], dst3[:, :, :], mb, ALU.is_equal))
            ib = rap(iotb[:], [[pi, NR], [0, NR], [1, 25]])
            E("v", lambda h, ib=ib: h.scalar_tensor_tensor(
                eqv[:, :, :], eqv[:, :, :], -BIG, ib, ALU.mult, ALU.add))
            E("v", lambda h, r=r: h.tensor_reduce(
                sel3[:, :, r:r + 1].rearrange("p j one -> p (j one)"),
                eqv[:, :, :], AXL.X, ALU.min))
            k1 = sel3[:, :, r:r + 1]
            k1b = rap(k1, [[k1.ap[0][0], NR], [8, NR], [0, 25]])
            ibr = rap(iotf[:], [[pi, NR], [0, NR], [1, 25]])
            E("v", lambda h, ibr=ibr, k1b=k1b: h.tensor_tensor(eqv[:, :, :], ibr, k1b, ALU.is_equal))
            E("v", lambda h: h.scalar_tensor_tensor(
                dst3[:, :, :], eqv[:, :, :], BIGD, dst3[:, :, :], ALU.mult, ALU.add))

        # ---------------- stage 3: gather ----------------
        eqk = dstt[:, 0:504].rearrange("p (j k) -> p j k", k=8)
        p_big = big[:].ap[0][0]
        p_shf = shf[:].ap[0][0]
        p_pat = pat[:].ap[0][0]
        p_dst = dstt[:].ap[0][0]
        E("v", lambda h: h.memset(grpT, 0.0))
        for oy, dy in enumerate(OFFS_G):
            for ox, dx in enumerate(OFFS_G):
                s_idx = oy * 5 + ox
                use_pat = (dy == 0 and dx == 0)
                if not use_pat:
                    Wt("s", "v")
                    emit_shift_load(dy, dx, shf3)
                    Wt("v", "s")
                E("v", lambda h, s_idx=s_idx: h.tensor_scalar(
                    eqk[:, :, :], sel3[:, :, :], float(s_idx), None, ALU.is_equal))
                for k in range(8):
                    srcn = (pat[:] if use_pat else shf[:]).rearrange("p (j w) -> p j w", w=64)
                    mbk = rap(dstt[:, 0:1], [[p_dst, NR], [8, NR], [0, 64]], doff=k)
                    tmpn = scr2[:].rearrange("p (j w) -> p j w", w=64)
                    E("v", lambda h, tmpn=tmpn, srcn=srcn, mbk=mbk: h.tensor_tensor(tmpn, srcn, mbk, ALU.mult))
                    gk = rap(grpT, [[p_big, NR], [8, NR], [504, 64]], doff=k)
                    E("v", lambda h, gk=gk, tmpn=tmpn: h.tensor_tensor(gk, gk, tmpn, ALU.add))

        Wt("s", "v")
        def f_grp_out(h):
            src = rap(grpT, [[p_big, NR], [504, 64], [1, 504]])
            dst = rap(grpD[:], [[504, NR], [RPAD, 64], [1, 504]])
            return h.dma_start(out=dst, in_=src)
        D_(f_grp_out)

        # ---------------- stage 4: transform ----------------
        def bf(h, src, dst, stride, sub):
            blk = 2 * stride
            npair = CH // blk
            se = rap(src, [[src.ap[0][0], 64], [blk, npair], [1, stride]])
            so = rap(se, se.ap, doff=stride)
            de = rap(dst, [[dst.ap[0][0], 64], [blk, npair], [1, stride]])
            do = rap(de, de.ap, doff=stride)
            if not sub:
                return h.tensor_tensor(de, se, so, ALU.add)
            return h.tensor_tensor(do, se, so, ALU.subtract)

        mk3 = rap(mk, [[mk.ap[0][0], 64], [8, 128], [1, 8]])
        for c in range(NCH):
            r0 = c * CH
            if c == 0:
                Wt("s", "s")
                Wt("v", "s")
            Wt("s", "v")
            em.barrier("s", "t")
            D_(lambda h, r0=r0: h.dma_start(out=gcol, in_=grpD[:, r0:r0 + CH]))
            Wt("t", "s")
            em.barrier("t", "v")
            E("t", lambda h: h.matmul(dct_ps[:, 0:512], KDDT_s, gcol[:, 0:512], start=True, stop=True))
            E("t", lambda h: h.matmul(dct_ps[:, 512:1024], KDDT_s, gcol[:, 512:1024], start=True, stop=True))
            Wt("v", "t")
            E("v", lambda h: h.tensor_copy(z1, dct_ps[:, :]))
            for (srcb, dstb, st) in ((z1, z2, 1), (z2, z1, 2), (z1, z2, 4)):
                E("v", lambda h, a=srcb, b=dstb, st=st: bf(h, a, b, st, False))
                E("v", lambda h, a=srcb, b=dstb, st=st: bf(h, a, b, st, True))
            E("v", lambda h: h.scalar_tensor_tensor(mk, z2, 1.0, z2, ALU.mult, ALU.mult))
            E("v", lambda h: h.tensor_scalar(mk, mk, (TAU * SQ8) ** 2, None, ALU.is_gt))
            E("v", lambda h: h.tensor_reduce(mkr_s, mk3, AXL.X, ALU.add))
            Wt("t", "v")
            E("t", lambda h: h.matmul(cnt_ps[:, :], ones_s, mkr_s, start=True, stop=True))
            Wt("v", "t")
            E("v", lambda h, c=c: h.tensor_copy(cnt_all[:, c * 128:(c + 1) * 128], cnt_ps[:, :]))
            E("v", lambda h: h.tensor_tensor(z2, z2, mk, ALU.mult))
            for (srcb, dstb, st) in ((z2, z1, 1), (z1, z2, 2), (z2, z1, 4)):
                E("v", lambda h, a=srcb, b=dstb, st=st: bf(h, a, b, st, False))
                E("v", lambda h, a=srcb, b=dstb, st=st: bf(h, a, b, st, True))
            Wt("t", "v")
            E("t", lambda h: h.matmul(inv_ps[:, 0:512], KDDTs_s, z1[:, 0:512], start=True, stop=True))
            E("t", lambda h: h.matmul(inv_ps[:, 512:1024], KDDTs_s, z1[:, 512:1024], start=True, stop=True))
            Wt("v", "t")
            E("v", lambda h: h.tensor_copy(recs, inv_ps[:, :]))
            Wt("s", "v")
            D_(lambda h, r0=r0: h.dma_start(out=recD[:, r0:r0 + CH], in_=recs))

        # ---------------- stage 5: weights ----------------
        Wt("s", "v")
        D_(lambda h: h.dma_start(out=cntD[:], in_=cnt_all))
        def f_w8(h):
            src = rap(cntD[:], [[63, NR], [1, NR]])
            return h.dma_start(out=w8[:], in_=src)
        D_(f_w8)
        Wt("v", "s")
        E("v", lambda h: h.tensor_scalar(w8[:], w8[:], 1.0, None, ALU.max))
        E("v", lambda h: h.reciprocal(w8[:], w8[:]))

        # ---------------- stage 6: rec_i + scatter ----------------
        em.barrier("s", "v")
        em.barrier("s", "t")
        def f_rec_in(h):
            src = rap(recD[:], [[504, NR], [RPAD, 64], [1, 504]])
            dst = rap(grpT, [[p_big, NR], [504, 64], [1, 504]])
            return h.dma_start(out=dst, in_=src)
        D_(f_rec_in)

        V3 = rap(pat[:], [[p_pat, NR], [64, NR], [1, 64]])
        As3 = rap(scr[:], [[scr[:].ap[0][0], NR], [64, NR], [1, 64]])
        A23 = rap(shf[:], [[p_shf, NR], [64, NR], [1, 64]])
        p_scr2 = scr2[:].ap[0][0]
        E("v", lambda h: h.memset(pat[:], 0.0))
        E("v", lambda h: h.memset(denV[:], 0.0))
        Wt("v", "s")

        for oy, dy in enumerate(OFFS_G):
            for ox, dx in enumerate(OFFS_G):
                s_idx = oy * 5 + ox
                E("v", lambda h, s_idx=s_idx: h.tensor_scalar(
                    eqk[:, :, :], sel3[:, :, :], float(s_idx), None, ALU.is_equal))
                E("v", lambda h: h.memset(scr[:], 0.0))
                for k in range(8):
                    reck = rap(grpT, [[p_big, NR], [8, NR], [504, 64]], doff=k)
                    mbk = rap(dstt[:, 0:1], [[p_dst, NR], [8, NR], [0, 64]], doff=k)
                    tmpv = rap(scr2[:], [[p_scr2, NR], [64, NR], [1, 64]])
                    E("v", lambda h, reck=reck, mbk=mbk, tmpv=tmpv: h.tensor_tensor(tmpv, reck, mbk, ALU.mult))
                    E("v", lambda h, tmpv=tmpv: h.tensor_tensor(As3, As3, tmpv, ALU.add))
                w8b = rap(w8[:], [[w8[:].ap[0][0], NR], [1, NR], [0, 64]])
                E("v", lambda h, w8b=w8b: h.tensor_tensor(As3, As3, w8b, ALU.mult))
                E("v", lambda h: h.tensor_reduce(dV1[:], eqk[:, :, :], AXL.X, ALU.add))
                E("v", lambda h: h.tensor_tensor(dV1[:], dV1[:], w8[:], ALU.mult))

                if dy == 0 and dx == 0:
                    E("v", lambda h: h.tensor_tensor(V3, V3, As3, ALU.add))
                    E("v", lambda h: h.tensor_tensor(denV[:], denV[:], dV1[:], ALU.add))
                    continue
                for (si0, di0, ni) in _pieces(dy):
                    for (sj0, dj0, nj) in _pieces(dx):
                        E("v", lambda h: h.memset(shf[:], 0.0))
                        E("v", lambda h: h.memset(dV2[:], 0.0))
                        Wt("s", "v")
                        def fmv(h, si0=si0, di0=di0, ni=ni, sj0=sj0, dj0=dj0, nj=nj):
                            return h.dma_start(
                                out=A23[di0:di0 + ni, dj0:dj0 + nj, :],
                                in_=As3[si0:si0 + ni, sj0:sj0 + nj, :])
                        D_(fmv)
                        def fmd(h, si0=si0, di0=di0, ni=ni, sj0=sj0, dj0=dj0, nj=nj):
                            return h.dma_start(
                                out=dV2[di0:di0 + ni, dj0:dj0 + nj],
                                in_=dV1[si0:si0 + ni, sj0:sj0 + nj])
                        D_(fmd)
                        Wt("v", "s")
                        E("v", lambda h: h.tensor_tensor(V3, V3, A23, ALU.add))
                        E("v", lambda h: h.tensor_tensor(denV[:], denV[:], dV2[:], ALU.add))

        # ---------------- stage 7: fold + finalize ----------------
        p_tmpO = tmpO.ap[0][0]
        p_dtm2 = dtm2.ap[0][0]
        E("v", lambda h: h.memset(O_t, 0.0))
        E("v", lambda h: h.memset(denF, 0.0))
        for pp in (0, 1):
            for qq in (0, 1):
                E("v", lambda h: h.memset(tmpO, 0.0))
                E("v", lambda h: h.memset(dtm2, 0.0))
                Wt("s", "v")
                for r in range(4):
                    a = 4 * pp + r
                    def ff(h, a=a, pp=pp, qq=qq, r=r):
                        src = rap(pat[:], [[p_pat, NR], [64, NR], [1, 4]],
                                  doff=a * 8 + 4 * qq)
                        dst = rap(tmpO[pp:pp + NR, :], [[p_tmpO, NR], [4, NR], [1, 4]],
                                  doff=r * 256 + qq * 4)
                        return h.dma_start(out=dst, in_=src)
                    D_(ff)
                def fd(h, pp=pp, qq=qq):
                    dst = rap(dtm2[pp:pp + NR, :], [[p_dtm2, NR], [1, NR]], doff=qq)
                    return h.dma_start(out=dst, in_=denV[:])
                D_(fd)
                Wt("v", "s")
                E("v", lambda h: h.tensor_tensor(O_t, O_t, tmpO, ALU.add))
                E("v", lambda h: h.tensor_tensor(denF, denF, dtm2, ALU.add))

        E("v", lambda h: h.tensor_scalar(drec, denF, 1e-8, None, ALU.max))
        E("v", lambda h: h.reciprocal(drec, drec))
        drecb = rap(drec, [[drec.ap[0][0], 64], [0, 4], [1, 64], [0, 4]])
        O4 = rap(O_t, [[O_t.ap[0][0], 64], [256, 4], [4, 64], [1, 4]])
        E("v", lambda h: h.tensor_tensor(O4, O4, drecb, ALU.mult))
        E("v", lambda h: h.tensor_scalar(m0, denF, 0.0, None, ALU.is_gt))
        m0b = rap(m0, [[m0.ap[0][0], 64], [0, 4], [1, 64], [0, 4]])
        E("v", lambda h: h.tensor_copy(imgG, imgU[:]))   # dequant fallback pixels
        E("v", lambda h: h.tensor_scalar(imgG, imgG, 1.0 / 65535.0, None, ALU.mult))
        E("v", lambda h: h.tensor_tensor(tmpO, O_t, imgG, ALU.subtract))
        tmpO4 = rap(tmpO, [[tmpO.ap[0][0], 64], [256, 4], [4, 64], [1, 4]])
        E("v", lambda h, tmpO4=tmpO4: h.tensor_tensor(tmpO4, tmpO4, m0b, ALU.mult))
        E("v", lambda h: h.tensor_tensor(imgG, imgG, tmpO, ALU.add))
        E("v", lambda h: h.tensor_copy(out16[:], imgG))  # f32 -> f16 for the cheap download leg
        Wt("s", "v")
        for r in range(4):
            def fo(h, r=r):
                src = rap(out16[:], [[out16[:].ap[0][0], 64], [4, 64], [1, 4]], doff=r * 256)
                dst = rap(out[:], [[1024, 64], [4, 64], [1, 4]], doff=r * 256)
                return h.dma_start(out=dst, in_=src)
            D_(fo)

        def replay(eng_name, h):
            for item in em.ops[eng_name]:
                if item[0] == "op":
                    inst = item[1](h)
                    inst.then_inc(sems[eng_name], step[eng_name])
                else:
                    _, p_, n = item
                    h.wait_ge(sems[p_], n * step[p_])

        @block.sync
        def _(s):
            replay("s", s)

        @block.vector
        def _(v):
            replay("v", v)

        @block.tensor
        def _(t):
            replay("t", t)

        @block.gpsimd
        def _(g):
            replay("g", g)

    return nc


# ===================== device runtime (cached jit) =====================

_DEV = {"ok": False}


def _init_device():
    import jax
    from jax.sharding import Mesh, PartitionSpec
    from jax.experimental.shard_map import shard_map
    from concourse import mybir
    from concourse.bass2jax import _bass_exec_p, partition_id_tensor, install_neuronx_cc_hook

    nc = _build_nc()
    install_neuronx_cc_hook()
    partition_name = nc.partition_id_tensor.name if nc.partition_id_tensor else None
    in_names, out_names, out_avals, zero_shapes = [], [], [], []
    for alloc in nc.m.functions[0].allocations:
        if not isinstance(alloc, mybir.MemoryLocationSet):
            continue
        name = alloc.memorylocations[0].name
        if alloc.kind == "ExternalInput":
            if name != partition_name:
                in_names.append(name)
        elif alloc.kind == "ExternalOutput":
            shape = tuple(alloc.tensor_shape)
            dtype = mybir.dt.np(alloc.dtype)
            out_names.append(name)
            out_avals.append(jax.core.ShapedArray(shape, dtype))
            zero_shapes.append((shape, dtype))
    n_params = len(in_names)
    n_outs = len(out_avals)
    all_names = in_names + out_names + ([partition_name] if partition_name else [])

    def _body(*args):
        operands = list(args)
        if partition_name is not None:
            operands.append(partition_id_tensor())
        return tuple(_bass_exec_p.bind(
            *operands, out_avals=tuple(out_avals), in_names=tuple(all_names),
            out_names=tuple(out_names), lowering_input_output_aliases=(),
            sim_require_finite=False, sim_require_nnan=False, nc=nc))

    devices = jax.devices()[:B]
    mesh = Mesh(np.asarray(devices), ("core",))
    specs_in = (PartitionSpec("core"),) * (n_params + n_outs)
    specs_out = (PartitionSpec("core"),) * n_outs
    donate = tuple(range(n_params, n_params + n_outs))
    fn = jax.jit(
        shard_map(_body, mesh=mesh, in_specs=specs_in, out_specs=specs_out, check_rep=False),
        donate_argnums=donate, keep_unused=True)

    from jax.sharding import NamedSharding
    shard = NamedSharding(mesh, PartitionSpec("core"))
    # cst is identical every call: keep it device-resident (no per-call upload)
    cst_dev = jax.device_put(np.concatenate([_make_cst()] * B, axis=0), shard)
    # donated output buffers are built ON DEVICE (broadcast-0, no upload) and
    # chained asynchronously into the main call
    import jax.numpy as jnp
    zfns = [
        jax.jit(lambda s=s, d=d: jnp.zeros((B * s[0],) + tuple(s[1:]), d),
                out_shardings=shard)
        for s, d in zero_shapes
    ]
    zeros_host = [np.zeros((B * s[0],) + s[1:], d) for s, d in zero_shapes]
    out_idx = out_names.index("out")

    def run(imgs):
        """imgs: [8, 256, 256] f32 -> [8, 256, 256] f32 (device BM3D)."""
        args = []
        for name in in_names:
            if name == "img":
                q = np.clip(np.rint(imgs.reshape(B * 256, 256) * 65535.0), 0, 65535)
                args.append(q.astype(np.uint16))
            elif name == "cst":
                args.append(cst_dev)
            else:
                raise KeyError(name)
        # donated output buffers are consumed each call; create fresh ones on
        # device (async) so no host->device zero upload happens
        args.extend(zf() for zf in zfns)
        outs = fn(*args)
        return np.asarray(outs[out_idx]).astype(np.float32).reshape(B, 256, 256)

    # warmup: triggers NEFF compile + executable load + device round trip
    run(np.zeros((B, 256, 256), np.float32))
    run(np.zeros((B, 256, 256), np.float32))
    _DEV["run"] = run
    _DEV["ok"] = True


try:
    _init_device()
except Exception:
    sys.stderr.write("device init failed; host fallback:\n" + traceback.format_exc())


# ===================== host fallback =====================

def _pre(img):
    Hp = H - P + 1
    pat = np.lib.stride_tricks.sliding_window_view(img, (P, P))
    r = np.arange(NR) * STRIDE
    c = np.clip(r[:, None] + OFFS[None, :], 0, Hp - 1)
    n_off = OFFS.size
    gy = np.broadcast_to(c[:, None, :, None], (NR, NR, n_off, n_off)).reshape(
        NR, NR, n_off * n_off)
    gx = np.broadcast_to(c[None, :, None, :], (NR, NR, n_off, n_off)).reshape(
        NR, NR, n_off * n_off)
    cand = pat[gy, gx]
    ref = pat[r[:, None], r[None, :]]
    dlt = cand - ref[:, :, None]
    dist = np.einsum("yxkab,yxkab->yxk", dlt, dlt)
    idx = np.argsort(dist, axis=-1, kind="stable")[..., :K].astype(np.int64)
    sy = np.take_along_axis(gy, idx, -1)
    sx = np.take_along_axis(gx, idx, -1)
    grp = np.take_along_axis(cand, idx[..., None, None], axis=2)
    X = np.zeros((NPAT, 64), np.float32)
    X[: NG * K] = grp.reshape(NG * K, 64)
    return X, sy, sx


def _transform_host(X):
    NGr = X.shape[0] // 8
    t = X.reshape(NGr, 8, 64).transpose(1, 0, 2).reshape(8, -1)
    t = (HD @ t).reshape(8, NGr * 64)
    tc = t.reshape(8 * NGr, 64) @ _KDD.T
    mask = np.abs(tc) > TAU
    pc = mask.sum(axis=1).astype(np.float32).reshape(8, NGr).T.reshape(-1)
    tpr = np.where(mask, tc, 0.0)
    z = tpr @ _KDD.T
    z = (HD @ z.reshape(8, NGr * 64)).reshape(8, NGr, 64)
    rec = z.transpose(1, 0, 2).reshape(-1, 64)
    return np.ascontiguousarray(rec, dtype=np.float32), pc


def _post(img, rec, pc, sy, sx):
    nnz = pc[: NG * K].reshape(NG, K).sum(axis=1).astype(np.float32)
    w = (1.0 / np.maximum(nnz, 1.0)).reshape(NR, NR)
    rec4 = rec[: NG * K].reshape(NR, NR, K, P, P)
    piy = sy[..., None] + np.arange(P)
    pix = sx[..., None] + np.arange(P)
    flat = (piy[..., :, None] * W + pix[..., None, :]).reshape(-1)
    vals = (rec4 * w[:, :, None, None, None]).reshape(-1)
    wv = np.broadcast_to(w[:, :, None, None, None], rec4.shape).reshape(-1)
    num = np.bincount(flat, weights=vals, minlength=H * W).astype(np.float32)
    den = np.bincount(flat, weights=wv, minlength=H * W).astype(np.float32)
    outp = num / np.maximum(den, 1e-8)
    return np.where(den > 0, outp, img.reshape(-1)).reshape(H, W).astype(np.float32)


def _host_bm3d(img):
    X, sy, sx = _pre(img)
    rec, pc = _transform_host(X)
    return _post(img, rec, pc, sy, sx)


# ===================== entry point =====================

def kernel(x):
    x = np.ascontiguousarray(np.asarray(x, dtype=np.float32))
    assert x.shape == (B, 1, H, W), x.shape
    if _DEV["ok"]:
        try:
            out = _DEV["run"](x.reshape(B, H, W))
            return out.reshape(B, 1, H, W)
        except Exception:
            sys.stderr.write("device run failed; host fallback:\n" + traceback.format_exc())
    result = np.empty((B, 1, H, W), np.float32)
    for i in range(B):
        result[i, 0] = _host_bm3d(x[i, 0])
    return result
